# revision 39
# baseline (speedup 1.0000x reference)
"""Trainium2 Bass kernel for single-head 2D attention (B=16, C=512, H=W=32).

Data-parallel over batch: 16 batch items / 8 cores = 2 per core. Weights
replicated. All matmuls run in fp8e4 (e4m3) with DoubleRow perf mode: each
PE pass contracts K=256 (two 128-blocks selected by the pair dim of a
[P, pairs, free] tile), giving ~2x the fp32r FLOP rate. PSUM accumulation
stays fp32; casts to fp8 ride existing ACT/DVE ops (bias-add, exp, copy).

  per batch item b (x_cn = x[b] viewed as [C, N=1024], channel-major):
    Qt[o,n] = cast8(sum_c wq8[c,o] x8[c,n] + bq[o])      (ACT bias+cast)
    Kt[o,n] = likewise via DVE tensor_scalar_add          (engine balance)
    V[n,o]  = cast8(sum_c x8[c,n] wv8[c,o])               (DVE copy)
    St[j,i] = sum_o Kt[o,j] Qt[o,i]
    E8[j,i] = cast8(exp(St/sqrt(C) - 2))   global shift: scores are
              O(+-6) and every query's row-max is >= 2.1, so e4m3's
              [2^-9, 240] window holds all weights that matter
    den[*,i]= sum_j E8[j,i]  via all-ones fp8 matmul (consistent with the
              quantized weights; sums over partitions AND broadcasts)
    svT[c,i]= cast8((sum_j V[j,c] E8[j,i]) * recip[i])
    y[c',n] = x32[c',n] + sum_c wo8[c,c'] sv8[c,n] + bo_eff[c']
  with bo_eff = bo + wo @ bv (V bias folded in on the host; exact because
  softmax rows sum to 1).
"""

import math

import numpy as np
import ml_dtypes

import concourse.mybir as mybir
import concourse.tile as tile
from concourse import bacc, bass_utils

B, C, H, W = 16, 512, 32, 32
N = H * W           # 1024 tokens
NCORES = 8
BPC = B // NCORES   # batch items per core
P = 128
CO = C // P         # 4 channel chunks
NB = N // 512       # 2 psum-bank slices of the token dim
NT = N // P         # 8 token chunks
F8 = ml_dtypes.float8_e4m3
SHIFT = 2.0         # softmax global shift (see module docstring)

_CACHE: dict = {}


def _build(reps: int = 1, variant: str = "defo"):
    lag0, lag1 = 4, 5
    k_on_act = False
    if variant.endswith("KA"):
        variant = variant[:-2]
        k_on_act = True
    y_gpsimd = False
    if variant.endswith("YG"):
        variant = variant[:-2]
        y_gpsimd = True
    k_split = False
    if variant.endswith("KB"):
        variant = variant[:-2]
        k_split = True
    out_il = False
    if variant.endswith("OC"):
        variant = variant[:-2]
        out_il = True
    v_split = False
    if variant.endswith("VB"):
        variant = variant[:-2]
        v_split = True
    deep_bufs = False
    if variant.endswith("BU"):
        variant = variant[:-2]
        deep_bufs = True
    pw = False
    if variant.endswith("PW"):
        variant = variant[:-2]
        pw = True
    warm_mode = "f32x8"
    for wm in ("W0", "WB", "W4"):
        if variant.endswith(wm):
            variant = variant[:-2]
            warm_mode = wm
    if "L" in variant:
        variant, lg = variant.split("L", 1)
        lag0, lag1 = int(lg[0]), int(lg[1])
    ab_groups = set()
    if variant.startswith("ab") and "_" in variant:
        pre, variant = variant.split("_", 1)
        ab_groups = {"qkv", "exp", "tail"} if pre == "ab" else {pre[2:]}
    no_ydma = "nodma" in ab_groups
    ab_qkv = "qkv" in ab_groups
    ab_exp = "exp" in ab_groups
    ab_tail = "tail" in ab_groups
    f32 = mybir.dt.float32
    f8 = mybir.dt.float8e4
    Ident = mybir.ActivationFunctionType.Identity
    Exp = mybir.ActivationFunctionType.Exp
    add = mybir.AluOpType.add
    DR = mybir.MatmulPerfMode.DoubleRow

    nc = bacc.Bacc("TRN2", debug=False, enable_asserts=False, num_devices=NCORES)
    x8_d = nc.dram_tensor("x8", (BPC, C, N), f8, kind="ExternalInput").ap()
    x32_d = nc.dram_tensor("x32", (BPC, C, N), f32, kind="ExternalInput").ap()
    w_d = {
        k: nc.dram_tensor(f"w{k}8", (C, C), f8, kind="ExternalInput").ap()
        for k in ("q", "k", "v", "o")
    }
    bq_d = nc.dram_tensor("bq", (P, CO), f32, kind="ExternalInput").ap()
    bk_d = nc.dram_tensor("bk", (P, CO), f32, kind="ExternalInput").ap()
    bo_d = nc.dram_tensor("bo", (P, CO), f32, kind="ExternalInput").ap()
    ones_d = nc.dram_tensor("ones8", (P, 2, P), f8, kind="ExternalInput").ap()
    y_d = nc.dram_tensor("y", (BPC, C, N), f32, kind="ExternalOutput").ap()

    with tile.TileContext(nc) as tc:
        with (
            tc.tile_pool(name="wp", bufs=1) as wp,
            tc.tile_pool(name="xp", bufs=2) as xp,
            tc.tile_pool(name="qkp", bufs=3 if deep_bufs else 2) as qkp,
            tc.tile_pool(name="vp", bufs=3 if deep_bufs else 2) as vp,
            tc.tile_pool(name="ep", bufs=3 if deep_bufs else 2) as ep,
            tc.tile_pool(name="svp", bufs=3 if deep_bufs else 2) as svp,
            tc.tile_pool(name="rp", bufs=3 if deep_bufs else 2) as rp,
            tc.tile_pool(name="yp", bufs=10 if deep_bufs else 6) as yp,
            tc.tile_pool(name="y32p", bufs=6) as y32p,
            tc.tile_pool(name="ps", bufs=3, space="PSUM") as ps,
            tc.tile_pool(name="ps2", bufs=2, space="PSUM") as ps2,
            tc.tile_pool(name="pssv", bufs=4, space="PSUM") as pssv,
            tc.tile_pool(name="psden", bufs=1, space="PSUM") as psden,
        ):
            # During the projection and output phases the attention pools
            # (pssv/psden) are idle, so per-chunk psums rotate across all 8
            # banks; the out cycle leads with ps+den so the nb=0 output half
            # is not gated on sv-ring buffers still held by ib1.
            proj_cycle = [(ps, "ps")] * 3 + [(pssv, "sv")] * 4 + [(psden, "den")]
            out_cycle = [(ps, "ps")] * 3 + [(psden, "den")] + [(pssv, "sv")] * 4
            rot_state = {"i": 0, "cycle": proj_cycle}

            def rot_reset(cycle):
                rot_state["i"] = 0
                rot_state["cycle"] = cycle

            class _Half:
                def __init__(self, t, h):
                    self.t, self.h = t, h

                def __getitem__(self, key):
                    if key == slice(None):
                        return self.t[:, self.h, :]
                    return self.t[:, self.h, key[1]]

            pw_state = {"i": 0, "tile": None}

            def rot_ps_pw(name="pt"):
                i = pw_state["i"] % 8
                pw_state["i"] += 1
                if i < 4:
                    if i % 2 == 0:
                        pw_state["tile"] = ps2.tile([P, 2, 512], f32,
                                                    tag="pp", name=name)
                    return _Half(pw_state["tile"], i % 2)
                return _Half2(pssv.tile([P, 512], f32, tag="sv", name=name))

            class _Half2:
                def __init__(self, t):
                    self.t = t

                def __getitem__(self, key):
                    if key == slice(None):
                        return self.t[:]
                    return self.t[:, key[1]]

            def rot_ps(name="pt"):
                if pw:
                    return rot_ps_pw(name)
                pool, tag = rot_state["cycle"][rot_state["i"] % 8]
                rot_state["i"] += 1
                return pool.tile([P, 512], f32, tag=tag, name=name)
            wt = {
                k: wp.tile([P, CO, C], f8, tag=f"w{k}", name=f"w{k}")
                for k in ("q", "k", "v", "o")
            }
            x8_tiles = [
                xp.tile([P, CO, N], f8, tag="x8", name=f"x8_{b}")
                for b in range(BPC)
            ]
            x32_tiles = [
                xp.tile([P, CO, N], f32, tag="x32", name=f"x32_{b}")
                for b in range(BPC)
            ]
            w_r = {
                k: w_d[k].rearrange("(co p) o -> p co o", p=P)
                for k in ("q", "k", "v", "o")
            }
            x8_r = [x8_d[b].rearrange("(ci p) n -> p ci n", p=P) for b in range(BPC)]
            x32_r = [x32_d[b].rearrange("(ci p) n -> p ci n", p=P) for b in range(BPC)]

            # PE warm-up on a memset tile (no DMA dependency): keeps the
            # p-state ramp going while the first wk/x chunks stream in.
            negc_t = wp.tile([P, 1], f32, tag="negc")
            nc.vector.memset(negc_t[:], -SHIFT)
            if warm_mode != "W0":
                wdt = mybir.dt.bfloat16 if warm_mode in ("WB", "W4") else f32
                nwarm = 4 if warm_mode == "W4" else 8
                warm_t = wp.tile([P, P], wdt, tag="warm_t")
                nc.vector.memset(warm_t[:], 0.0)
                for i in range(nwarm):
                    wpt = rot_ps(name=f"warm{i}")
                    nc.tensor.matmul(wpt[:, 0:P], warm_t[:], warm_t[:],
                                     start=True, stop=True)

            # loads in first-consumption order: the projection interleave
            # consumes K, then Q, then V chunk-wise
            for ci in range(CO):
                nc.sync.dma_start(wt["k"][:, ci], w_r["k"][:, ci])
                nc.sync.dma_start(x8_tiles[0][:, ci], x8_r[0][:, ci])
            bk_t = wp.tile([P, CO], f32, tag="bk")
            nc.sync.dma_start(bk_t[:], bk_d)
            for ci in range(CO):
                nc.sync.dma_start(wt["q"][:, ci], w_r["q"][:, ci])
            bq_t = wp.tile([P, CO], f32, tag="bq")
            nc.sync.dma_start(bq_t[:], bq_d)
            for ci in range(CO):
                nc.sync.dma_start(wt["v"][:, ci], w_r["v"][:, ci])
            ones_t = wp.tile([P, 2, P], f8, tag="ones")
            nc.sync.dma_start(ones_t[:], ones_d)
            for ci in range(CO):
                nc.sync.dma_start(wt["o"][:, ci], w_r["o"][:, ci])
            bo_t = wp.tile([P, CO], f32, tag="bo")
            nc.sync.dma_start(bo_t[:], bo_d)
            for ci in range(CO):
                nc.sync.dma_start(x8_tiles[1][:, ci], x8_r[1][:, ci])
            # residual inputs are only needed at the very end of each item
            for b in range(BPC):
                for ci in range(CO):
                    nc.sync.dma_start(x32_tiles[b][:, ci], x32_r[b][:, ci])

            inv_sqrt_c = 1.0 / math.sqrt(C)

            defer_out = variant == "defo"
            pending_out = []

            def out_one(sv_, b_, nb, c2, pt):
                        for cp in range(0, CO, 2):
                            nc.tensor.matmul(
                                pt[:],
                                wt["o"][:, cp:cp + 2, c2 * P:(c2 + 1) * P],
                                sv_[:, cp:cp + 2, nb * 512:(nb + 1) * 512],
                                start=(cp == 0), stop=(cp == CO - 2),
                                perf_mode=DR,
                            )
                        yt = yp.tile([P, 512], f32, tag="y")
                        w_ = 1 if ab_tail else 512
                        if y_gpsimd:
                            # ACT lifts the psum to SBUF; gpsimd (idle, but
                            # PSUM-blind) does the bias+residual combine,
                            # keeping the deferred y-burst off DVE
                            y32 = y32p.tile([P, 512], f32, tag="y32")
                            nc.scalar.activation(y32[:, 0:w_], pt[:, 0:w_],
                                                 Ident,
                                                 bias=bo_t[:, c2:c2 + 1])
                            nc.gpsimd.tensor_add(
                                yt[:, 0:w_], y32[:, 0:w_],
                                x32_tiles[b_][:, c2, nb * 512:nb * 512 + w_],
                            )
                        else:
                            nc.vector.scalar_tensor_tensor(
                                yt[:, 0:w_], pt[:, 0:w_], bo_t[:, c2:c2 + 1],
                                x32_tiles[b_][:, c2, nb * 512:nb * 512 + w_],
                                add, add,
                            )
                        if not no_ydma:
                            nc.sync.dma_start(
                                y_d[b_, c2 * P:(c2 + 1) * P,
                                    nb * 512:(nb + 1) * 512],
                                yt[:],
                            )

            def out_chunks_for(sv_, b_, skip_nb0=False):
                rot_reset(out_cycle)
                for nb in range(NB):
                    for c2 in range(CO):
                        if skip_nb0 and nb == 0:
                            continue
                        out_one(sv_, b_, nb, c2, rot_ps())

            items = [i for _ in range(reps) for i in range(BPC)]
            for b in items:
                x8_sb = x8_tiles[b]

                qt = qkp.tile([P, CO, N], f8, tag="qt")
                kt = qkp.tile([P, CO, N], f8, tag="kt")
                v_sb = vp.tile([P, NT, C], f8, tag="v")

                def q_chunk(oc, nb):
                    pt = rot_ps()
                    for cp in range(0, CO, 2):
                        nc.tensor.matmul(
                            pt[:],
                            wt["q"][:, cp:cp + 2, oc * P:(oc + 1) * P],
                            x8_sb[:, cp:cp + 2, nb * 512:(nb + 1) * 512],
                            start=(cp == 0), stop=(cp == CO - 2),
                            perf_mode=DR,
                        )
                    w_ = 1 if ab_qkv else 512
                    nc.scalar.activation(
                        qt[:, oc, nb * 512:nb * 512 + w_], pt[:, 0:w_],
                        Ident, bias=bq_t[:, oc:oc + 1])

                def k_chunk(oc, nb):
                    pt = rot_ps()
                    for cp in range(0, CO, 2):
                        nc.tensor.matmul(
                            pt[:],
                            wt["k"][:, cp:cp + 2, oc * P:(oc + 1) * P],
                            x8_sb[:, cp:cp + 2, nb * 512:(nb + 1) * 512],
                            start=(cp == 0), stop=(cp == CO - 2),
                            perf_mode=DR,
                        )
                    w_ = 1 if ab_qkv else 512
                    if k_on_act or (k_split and (oc + nb) % 2 == 0):
                        nc.scalar.activation(
                            kt[:, oc, nb * 512:nb * 512 + w_], pt[:, 0:w_],
                            Ident, bias=bk_t[:, oc:oc + 1])
                    else:
                        nc.vector.tensor_scalar_add(
                            kt[:, oc, nb * 512:nb * 512 + w_], pt[:, 0:w_],
                            bk_t[:, oc:oc + 1])

                def v_chunk(t8):
                    pt = rot_ps()
                    for cp in range(0, CO, 2):
                        nc.tensor.matmul(
                            pt[:],
                            x8_sb[:, cp:cp + 2, t8 * P:(t8 + 1) * P],
                            wt["v"][:, cp:cp + 2, :],
                            start=(cp == 0), stop=(cp == CO - 2),
                            perf_mode=DR,
                        )
                    w_ = 1 if ab_qkv else 512
                    if variant == "v1" or (v_split and t8 % 2 == 1):
                        nc.vector.tensor_copy(v_sb[:, t8, 0:w_], pt[:, 0:w_])
                    else:
                        nc.scalar.activation(v_sb[:, t8, 0:w_], pt[:, 0:w_],
                                             Ident)

                rot_reset(proj_cycle)
                if variant == "v1":
                    # stage-ordered projections
                    for oc in range(CO):
                        for nb in range(NB):
                            q_chunk(oc, nb)
                    for t8 in range(NT):
                        v_chunk(t8)
                    for oc in range(CO):
                        for nb in range(NB):
                            k_chunk(oc, nb)
                else:
                    # Interleave the three projections chunk-wise: K lands on
                    # DVE, Q/V on ACT, so the PE stream (6 matmuls per
                    # triple, ~900ns) covers both casters. With out_il the
                    # previous item's nb0 output chunks ride the back half of
                    # the stream, spreading the deferred y-burst on DVE.
                    prev = pending_out.pop(0) if pending_out else None
                    for s in range(NT):
                        oc, nb = s % CO, s // CO
                        k_chunk(oc, nb)
                        q_chunk(oc, nb)
                        v_chunk(s)
                        if out_il and prev is not None and s >= 4:
                            out_one(prev[0], prev[1], 0, s - 4, rot_ps())
                    if prev is not None:
                        out_chunks_for(*prev, skip_nb0=out_il)
                    prev = None

                if pending_out:
                    out_chunks_for(*pending_out.pop(0))

                est = ep.tile([P, NT, N], f8, tag="est")
                recip = rp.tile([P, N], f32, tag="recip")
                sv = svp.tile([P, CO, N], f8, tag="sv")

                def score_chunk(ib, jc):
                    ibs = slice(ib * 512, (ib + 1) * 512)
                    pt = (rot_ps() if variant == "v1" else
                          ps.tile([P, 512], f32, tag="ps", name="pt"))
                    for op_ in range(0, CO, 2):
                        nc.tensor.matmul(
                            pt[:],
                            kt[:, op_:op_ + 2, jc * P:(jc + 1) * P],
                            qt[:, op_:op_ + 2, ibs],
                            start=(op_ == 0), stop=(op_ == CO - 2),
                            perf_mode=DR,
                        )
                    w_ = 1 if ab_exp else 512
                    nc.scalar.activation(
                        est[:, jc, ib * 512:ib * 512 + w_], pt[:, 0:w_],
                        Exp, bias=negc_t[:, 0:1], scale=inv_sqrt_c,
                    )

                if variant == "v1":
                    # stage-ordered attention: scores, then den+recip, then SV
                    for jc in range(NT):
                        for ib in range(NB):
                            score_chunk(ib, jc)
                    for ib in range(NB):
                        ibs = slice(ib * 512, (ib + 1) * 512)
                        dpt = rot_ps()
                        for jp in range(0, NT, 2):
                            nc.tensor.matmul(
                                dpt[:], ones_t[:], est[:, jp:jp + 2, ibs],
                                start=(jp == 0), stop=(jp == NT - 2),
                                perf_mode=DR,
                            )
                        nc.vector.reciprocal(recip[:, ibs], dpt[:])
                    for cc in range(CO):
                        for ib in range(NB):
                            ibs = slice(ib * 512, (ib + 1) * 512)
                            pt = rot_ps()
                            for jp in range(0, NT, 2):
                                nc.tensor.matmul(
                                    pt[:],
                                    v_sb[:, jp:jp + 2, cc * P:(cc + 1) * P],
                                    est[:, jp:jp + 2, ibs],
                                    start=(jp == 0), stop=(jp == NT - 2),
                                    perf_mode=DR,
                                )
                            nc.vector.tensor_mul(
                                sv[:, cc, ibs], pt[:], recip[:, ibs])
                else:
                    # den/SV accumulation pipelined behind the score/exp
                    # stream (per-ib or flattened across both ib halves)
                    denp = {}
                    svps = {}

                    def den_sv(ib, jp):
                        ibs = slice(ib * 512, (ib + 1) * 512)
                        if jp == 0:
                            # allocate at first-pair issue time: any earlier
                            # recycles the previous ib's banks while its
                            # accumulation groups are still open
                            denp[ib] = psden.tile([P, 512], f32, tag="den",
                                                  name=f"den{ib}")
                            svps[ib] = [
                                pssv.tile([P, 512], f32, tag="sv",
                                          name=f"svp{cc}")
                                for cc in range(CO)
                            ]
                        nc.tensor.matmul(
                            denp[ib][:], ones_t[:], est[:, jp:jp + 2, ibs],
                            start=(jp == 0), stop=(jp == NT - 2),
                            perf_mode=DR,
                        )
                        for cc in range(CO):
                            nc.tensor.matmul(
                                svps[ib][cc][:],
                                v_sb[:, jp:jp + 2, cc * P:(cc + 1) * P],
                                est[:, jp:jp + 2, ibs],
                                start=(jp == 0), stop=(jp == NT - 2),
                                perf_mode=DR,
                            )
                        if jp == NT - 2:
                            w_ = 1 if ab_tail else 512
                            ibw = slice(ib * 512, ib * 512 + w_)
                            nc.vector.reciprocal(recip[:, ibw],
                                                 denp[ib][:, 0:w_])
                            for cc in range(CO):
                                nc.vector.tensor_mul(
                                    sv[:, cc, ibw], svps[ib][cc][:, 0:w_],
                                    recip[:, ibw])

                    if pw:
                        for ib in range(NB):
                            ibs = slice(ib * 512, (ib + 1) * 512)
                            svps_l = [
                                pssv.tile([P, 512], f32, tag="sv",
                                          name=f"svp{cc}")
                                for cc in range(CO)
                            ]

                            def sv_acc(jp):
                                for cc in range(CO):
                                    nc.tensor.matmul(
                                        svps_l[cc][:],
                                        v_sb[:, jp:jp + 2,
                                             cc * P:(cc + 1) * P],
                                        est[:, jp:jp + 2, ibs],
                                        start=(jp == 0), stop=(jp == NT - 2),
                                        perf_mode=DR,
                                    )

                            for jp in range(0, NT, 2):
                                pt2 = ps2.tile([P, 2, 512], f32, tag="pp",
                                               name="sc")
                                for h in (0, 1):
                                    jc = jp + h
                                    for op_ in range(0, CO, 2):
                                        nc.tensor.matmul(
                                            pt2[:, h, :],
                                            kt[:, op_:op_ + 2,
                                               jc * P:(jc + 1) * P],
                                            qt[:, op_:op_ + 2, ibs],
                                            start=(op_ == 0),
                                            stop=(op_ == CO - 2),
                                            perf_mode=DR,
                                        )
                                nc.scalar.activation(
                                    est[:, jp:jp + 2, ibs], pt2[:],
                                    Exp, bias=negc_t[:, 0:1],
                                    scale=inv_sqrt_c,
                                )
                                if jp >= 2:
                                    sv_acc(jp - 2)
                            sv_acc(NT - 2)
                            dpt2 = ps2.tile([P, 2, 512], f32, tag="pp",
                                            name="dn")
                            for jp2 in range(0, NT, 2):
                                nc.tensor.matmul(
                                    dpt2[:, 0, :], ones_t[:],
                                    est[:, jp2:jp2 + 2, ibs],
                                    start=(jp2 == 0), stop=(jp2 == NT - 2),
                                    perf_mode=DR,
                                )
                            nc.vector.reciprocal(recip[:, ibs],
                                                 dpt2[:, 0, :])
                            for cc in range(CO):
                                nc.vector.tensor_mul(
                                    sv[:, cc, ibs], svps_l[cc][:],
                                    recip[:, ibs])
                    elif variant == "perib":
                        for ib in range(NB):
                            for jc in range(NT):
                                score_chunk(ib, jc)
                                if jc >= 3 and jc % 2 == 1:
                                    den_sv(ib, jc - 3)
                            den_sv(ib, NT - 2)
                    else:  # flat
                        seq = [(ib, jc) for ib in range(NB) for jc in range(NT)]
                        # pair (ib, jp) is ready after score chunk
                        # t=ib*NT+jp+1; issue it `lag` chunks later (deeper
                        # for ib1 so its den bank recycles through recip).
                        issue_at = {}
                        for ib in range(NB):
                            for jp in range(0, NT, 2):
                                lag = lag0 if ib == 0 else lag1
                                issue_at.setdefault(
                                    min(ib * NT + jp + 1 + lag, len(seq)), []
                                ).append((ib, jp))
                        for t, (ib, jc) in enumerate(seq):
                            score_chunk(ib, jc)
                            if t + 1 < len(seq):
                                for pib, pjp in issue_at.get(t + 1, []):
                                    den_sv(pib, pjp)

                        for pib, pjp in issue_at.get(len(seq), []):
                            den_sv(pib, pjp)

                # --- output projection + bias + residual. In the "defo"
                #     variant each item's output is emitted after the NEXT
                #     item's projections, so the sv-mul DVE tail drains
                #     behind projection matmuls instead of stalling the PE.
                if defer_out:
                    pending_out.append((sv, b))
                else:
                    out_chunks_for(sv, b)
            if pending_out:
                out_chunks_for(*pending_out.pop(0))
    nc.compile()
    return nc


def _prep_inputs(inputs):
    x = np.asarray(inputs["x"], np.float32).reshape(B, C, N)
    x8 = x.astype(F8)
    wts = {}
    for k in ("q", "k", "v", "o"):
        wts[f"w{k}8"] = np.ascontiguousarray(
            np.asarray(inputs[f"w{k}"], np.float32).T).astype(F8)
    bq = np.asarray(inputs["bq"], np.float32)
    bk = np.asarray(inputs["bk"], np.float32)
    bv = np.asarray(inputs["bv"], np.float32)
    bo = np.asarray(inputs["bo"], np.float32)
    wo = np.asarray(inputs["wo"], np.float32)
    bo_eff = bo + wo @ bv

    def per_part(v):  # [C] -> [P, CO]
        return np.ascontiguousarray(v.reshape(CO, P).T)

    shared = {
        **wts,
        "bq": per_part(bq),
        "bk": per_part(bk),
        "bo": per_part(bo_eff),
        "ones8": np.ones((P, 2, P), F8),
    }
    in_maps = [
        {
            **shared,
            "x8": np.ascontiguousarray(x8[i * BPC:(i + 1) * BPC]),
            "x32": np.ascontiguousarray(x[i * BPC:(i + 1) * BPC]),
        }
        for i in range(NCORES)
    ]
    return in_maps


def _make_axon_runner(nc):
    """Cached jitted shard_map runner for the axon/PJRT path, so repeated
    kernel() calls execute without re-tracing (the stock
    run_bass_kernel_spmd path builds a fresh jit closure per call)."""
    import jax
    from jax.sharding import Mesh, NamedSharding, PartitionSpec

    import warnings

    with warnings.catch_warnings():
        warnings.simplefilter("ignore")
        from jax.experimental.shard_map import shard_map

    import concourse.bass2jax as b2j

    b2j.install_neuronx_cc_hook()
    partition_name = nc.partition_id_tensor.name if nc.partition_id_tensor else None
    in_names, out_names, out_avals = [], [], []
    for alloc in nc.m.functions[0].allocations:
        if not isinstance(alloc, mybir.MemoryLocationSet):
            continue
        name = alloc.memorylocations[0].name
        if alloc.kind == "ExternalInput":
            if name != partition_name:
                in_names.append(name)
        elif alloc.kind == "ExternalOutput":
            out_names.append(name)
            out_avals.append(
                jax.core.ShapedArray(tuple(alloc.tensor_shape),
                                     mybir.dt.np(alloc.dtype)))
    n_params = len(in_names)
    bind_in_names = list(in_names) + list(out_names)
    if partition_name is not None:
        bind_in_names.append(partition_name)

    def _body(*args):
        operands = list(args)
        if partition_name is not None:
            operands.append(b2j.partition_id_tensor())
        return tuple(b2j._bass_exec_p.bind(
            *operands,
            out_avals=tuple(out_avals),
            in_names=tuple(bind_in_names),
            out_names=tuple(out_names),
            lowering_input_output_aliases=(),
            sim_require_finite=True,
            sim_require_nnan=True,
            nc=nc,
        ))

    devices = jax.devices()[:NCORES]
    mesh = Mesh(np.asarray(devices), ("core",))
    n_outs = len(out_avals)
    fn = jax.jit(
        shard_map(_body, mesh=mesh,
                  in_specs=(PartitionSpec("core"),) * (n_params + n_outs),
                  out_specs=(PartitionSpec("core"),) * n_outs,
                  check_rep=False),
        keep_unused=True,
    )
    sharding = NamedSharding(mesh, PartitionSpec("core"))
    zeros = [
        np.zeros((NCORES * a.shape[0], *a.shape[1:]), a.dtype) for a in out_avals
    ]
    dev_zeros = [jax.device_put(z, sharding) for z in zeros]

    def run(in_maps):
        concat_in = [
            np.concatenate([np.asarray(m[nm]) for m in in_maps], axis=0)
            for nm in in_names
        ]
        dev_in = [jax.device_put(a, sharding) for a in concat_in]
        outs = fn(*dev_in, *dev_zeros)
        return [
            {nm: np.asarray(outs[i]).reshape(NCORES, *out_avals[i].shape)[c]
             for i, nm in enumerate(out_names)}
            for c in range(NCORES)
        ]

    return run


def kernel(**inputs) -> np.ndarray:
    if "nc" not in _CACHE:
        _CACHE["nc"] = _build()
    nc = _CACHE["nc"]
    in_maps = _prep_inputs(inputs)

    from concourse._compat import axon_active

    if axon_active():
        if "runner" not in _CACHE:
            _CACHE["runner"] = _make_axon_runner(nc)
        results = _CACHE["runner"](in_maps)
    else:
        results = bass_utils.run_bass_kernel_spmd(
            nc, in_maps, core_ids=list(range(NCORES))).results
    y = np.concatenate([r["y"] for r in results], axis=0)
    return y.reshape(B, C, H, W)
